# revision 1
# baseline (speedup 1.0000x reference)
"""Bidirectional Mamba (PartContextMamba) Trainium2 Bass kernel.

Sharding: pure data parallelism over batch (1024 -> 8 cores x 128 batch).
Per core (feature-major, batch*L = 768 tokens on the free dim; token slice
[h*384,(h+1)*384) == batch half h since tokens are (b,l) l-inner):

  xT [768d, 768tok] fp16 (PE transpose of the x shard)
  per direction (fwd, bwd; bwd realized via time-reversed APs):
    xi = W_in_xi @ xT                    (PE fp16, f32 accum)
    xc = silu(causal dwconv + b)         (DVE taps, ACT silu) fp16
    x_dbl = W_xp @ xc -> dt_lo[48] f32, B[16], C[16] fp16
    per b-half h, per d-tile:
      dt = softplus(W_dt @ dt_lo + dt_b)   f32
      scan-space (n, b, t), t innermost:
        POW[n,b,t] = exp(-(n+1)*dt)        (16 ACT exps; dt[t=0] poisoned ->
                                            POW=0 => per-(n,b) reset)
        WB = (dt*xc) (x) B_rep             (DVE TT fp16)
        h = tensor_tensor_scan(POW, WB)    (in-place over POW)
        ycum = custom DVE scan(ADD, h*C_rep) iterated (b,t,n); y = boundary
        diffs of ycum at n=15; y += D*xc
      ygated = y * silu(W_z @ xT)          (token order; bwd un-reverses)
    yout += W_out @ ygated               (PE, PSUM k-accumulation)
  out = LayerNorm(x + yout^T)            (PE transpose, token-major)

B/C replication across partitions: SBUF->DRAM contiguous stage, then
DRAM->SBUF partition-broadcast read (step-0 partition AP).
"""

import re

import numpy as np

_CACHE: dict = {}

B = 128          # batch per core
L = 6
D = 768
DI = 1536
NT = 12          # d-tiles
NS = 16          # ssm states
R = 48           # dt rank
TOK = B * L
ET = 6           # e-tiles / token-tiles
KT = 6           # k-tiles of D
NH = 2           # b-halves
BH = B // NH     # 64
NQ = 2           # ycum quarters per half
BQ = BH // NQ    # 32
HL = BH * L      # 384 (tokens per half)
HV = NS * HL     # 6144
QBT = BQ * L     # 192


def _register_mulcumsum():
    import concourse.dve_ops as dve_ops
    from concourse.dve_spec import Spec, Src0, Src1, AluOp, scan as dve_scan

    name = "MULCUMSUM_ANT"
    for o in dve_ops.OPS:
        if o.name == name:
            return o

    def ref(in0, in1, s0, s1, imm2):
        a = in0.astype(np.float32) * in1.astype(np.float32)
        flat = a.reshape(a.shape[0], -1)
        return np.cumsum(flat, axis=1).reshape(a.shape)

    spec = Spec(body=dve_scan(AluOp.ADD, Src0 * Src1), reference=ref)
    op = dve_ops.DveOp(name, spec, subdim=False, uops_sha={})
    dve_ops.OPS.append(op)
    dve_ops._SUB_OPCODE_FOR_NAME[name] = max(dve_ops._SUB_OPCODE_FOR_NAME.values()) + 1
    dve_ops.CUSTOM_DVE_SPECS[name] = spec
    for ver in ("v3", "v4"):
        try:
            op.compile(ver)
        except ValueError as e:
            m = re.search(rf"{ver}: ([0-9a-f]+)", str(e))
            assert m, f"could not parse uops sha from: {e}"
            op.uops_sha[ver] = m.group(1)
        op.compile(ver)
    return op


def _build_module(debug=False):
    import concourse.bass as bass
    import concourse.bacc as bacc
    import concourse.mybir as mybir
    import concourse.tile as tile
    from concourse.masks import make_identity

    MULCUMSUM = _register_mulcumsum()

    f32 = mybir.dt.float32
    f16 = mybir.dt.float16
    AP = bass.AP
    AF = mybir.ActivationFunctionType
    OP = mybir.AluOpType

    nc = bacc.Bacc("TRN2", target_bir_lowering=False)

    x_d = nc.dram_tensor("x", [TOK, D], f32, kind="ExternalInput")
    ins = {}
    for d in ("f", "b"):
        ins[f"win_{d}"] = nc.dram_tensor(f"win_{d}", [D, DI], f16, kind="ExternalInput")
        ins[f"wz_{d}"] = nc.dram_tensor(f"wz_{d}", [NT, 128, KT, 128], f16, kind="ExternalInput")
        ins[f"wxp_{d}"] = nc.dram_tensor(f"wxp_{d}", [128, NT, 80], f16, kind="ExternalInput")
        ins[f"wdt_{d}"] = nc.dram_tensor(f"wdt_{d}", [R, DI], f32, kind="ExternalInput")
        ins[f"wout_{d}"] = nc.dram_tensor(f"wout_{d}", [DI, D], f16, kind="ExternalInput")
        ins[f"aux_{d}"] = nc.dram_tensor(f"aux_{d}", [DI, 8], f32, kind="ExternalInput")
    lng_d = nc.dram_tensor("ln_g", [D], f32, kind="ExternalInput")
    lnb_d = nc.dram_tensor("ln_b", [D], f32, kind="ExternalInput")
    out_d = nc.dram_tensor("out", [TOK, D], f32, kind="ExternalOutput")

    def dram_ap(t, offset, ap):
        return AP(tensor=t, offset=offset, ap=ap)

    def dbg(name, ap):
        if not debug:
            return
        p = ap.partition_size()
        counts = [c for _, c in ap.ap[1:]]
        flat = 1
        for c in counts:
            flat *= c
        t = nc.dram_tensor(f"dbg_{name}", [p] + counts, ap.dtype,
                           kind="ExternalOutput")
        nc.sync.dma_start(t[:], ap)

    with tile.TileContext(nc) as tc:
        with (
            tc.tile_pool(name="consts", bufs=1) as consts,
            tc.tile_pool(name="persist", bufs=1) as persist,
            tc.tile_pool(name="wpool", bufs=1) as wpool,
            tc.tile_pool(name="wstream", bufs=2) as wstream,
            tc.tile_pool(name="tr2", bufs=2) as tr2,
            tc.tile_pool(name="tr1", bufs=1) as tr1,
            tc.tile_pool(name="scanp", bufs=2) as scanp,
            tc.tile_pool(name="reps", bufs=1) as repsp,
            tc.tile_pool(name="dram", bufs=1, space="DRAM") as dramp,
            tc.tile_pool(name="psA", bufs=2, space="PSUM") as psA,
            tc.tile_pool(name="psT", bufs=2, space="PSUM") as psT,
            tc.tile_pool(name="psO", bufs=1, space="PSUM") as psO,
        ):
            # ---------------- constants ----------------
            ident = consts.tile([128, 128], f32)
            make_identity(nc, ident)
            identh = consts.tile([128, 128], f16)
            nc.vector.tensor_copy(identh[:], ident[:])
            g_rep = consts.tile([128, D], f32)
            nc.sync.dma_start(g_rep[:], dram_ap(lng_d, 0, [[0, 128], [1, D]]))
            b_rep = consts.tile([128, D], f32)
            nc.sync.dma_start(b_rep[:], dram_ap(lnb_d, 0, [[0, 128], [1, D]]))
            eps_t = consts.tile([128, 1], f32)
            nc.vector.memset(eps_t[:], 1e-5)
            aux = {}
            for d in ("f", "b"):
                aux[d] = consts.tile([128, NT, 8], f32, tag=f"aux_{d}", name=f"aux_{d}")
                nc.sync.dma_start(
                    aux[d][:],
                    dram_ap(ins[f"aux_{d}"], 0, [[8, 128], [8 * 128, NT], [1, 8]]),
                )

            # ---------------- xT (fp16) via PE transpose ----------------
            xT = persist.tile([128, KT, TOK], f16, tag="xT")
            for tt in range(ET):
                xtok = tr1.tile([128, D], f32, tag="xtok")
                nc.sync.dma_start(xtok[:], x_d[tt * 128:(tt + 1) * 128, :])
                for ec in range(KT):
                    pst = psT.tile([128, 128], f32, tag="pst")
                    nc.tensor.transpose(pst[:], xtok[:, ec * 128:(ec + 1) * 128], ident[:])
                    nc.scalar.copy(xT[:, ec, tt * 128:(tt + 1) * 128], pst[:])

            dbg("xT", xT[:])
            yout = persist.tile([128, ET, TOK], f16, tag="yout")
            xc = persist.tile([128, NT, TOK], f16, tag="xc")

            for dir_i, d in enumerate(("f", "b")):
                fwd = d == "f"

                # ---------------- phase 1: in_proj + conv -> xc ------------
                win = wpool.tile([128, KT, DI], f16, tag="wbig")
                for kt in range(KT):
                    nc.sync.dma_start(
                        win[:, kt, :], ins[f"win_{d}"][kt * 128:(kt + 1) * 128, :]
                    )
                for mt in range(NT):
                    xi_t = tr2.tile([128, B, L], f16, tag="xi")
                    xi_f = xi_t[:].rearrange("p b l -> p (b l)")
                    for ng in range(2):
                        ps = psA.tile([128, 384], f32, tag="psA")
                        for kt in range(KT):
                            nc.tensor.matmul(
                                ps[:],
                                win[:, kt, mt * 128:(mt + 1) * 128],
                                xT[:, kt, ng * 384:(ng + 1) * 384],
                                start=(kt == 0),
                                stop=(kt == KT - 1),
                            )
                        nc.scalar.copy(xi_f[:, ng * 384:(ng + 1) * 384], ps[:])

                    acc = tr2.tile([128, B, L], f32, tag="fbl32")
                    cw = [aux[d][:, mt, k:k + 1] for k in range(4)]
                    xi_v = xi_t[:]
                    if fwd:
                        nc.vector.tensor_scalar(
                            out=acc[:], in0=xi_v, scalar1=cw[3], scalar2=None,
                            op0=OP.mult,
                        )
                        for k in range(3):
                            cnt = k + 3
                            o = acc[:, :, 3 - k:6]
                            nc.vector.scalar_tensor_tensor(
                                out=o, in0=xi_v[:, :, 0:cnt], scalar=cw[k],
                                in1=o, op0=OP.mult, op1=OP.add,
                            )
                    else:
                        rev_full = AP(
                            tensor=xi_v.tensor, offset=xi_v.offset + 5,
                            ap=[xi_v.ap[0], [L, B], [-1, L]],
                        )
                        nc.vector.tensor_scalar(
                            out=acc[:], in0=rev_full, scalar1=cw[3], scalar2=None,
                            op0=OP.mult,
                        )
                        for k in range(3):
                            cnt = k + 3
                            o = acc[:, :, 3 - k:6]
                            rev = AP(
                                tensor=xi_v.tensor, offset=xi_v.offset + 5,
                                ap=[xi_v.ap[0], [L, B], [-1, cnt]],
                            )
                            nc.vector.scalar_tensor_tensor(
                                out=o, in0=rev, scalar=cw[k],
                                in1=o, op0=OP.mult, op1=OP.add,
                            )
                    sgc = tr2.tile([128, B, L], f16, tag="sgc")
                    nc.scalar.activation(
                        out=sgc[:].rearrange("p b l -> p (b l)"),
                        in_=acc[:].rearrange("p b l -> p (b l)"),
                        func=AF.Sigmoid,
                        bias=aux[d][:, mt, 4:5],
                    )
                    nc.vector.scalar_tensor_tensor(
                        out=xc[:, mt, :].rearrange("p (b l) -> p b l", l=L),
                        in0=acc[:], scalar=aux[d][:, mt, 4:5], in1=sgc[:],
                        op0=OP.add, op1=OP.mult,
                    )

                dbg(f"xc_{d}", xc[:])
                # ---------------- phase 2: x_proj -> dt_lo, B, C -----------
                wxp = wpool.tile([128, NT, 80], f16, tag="wbig")
                nc.sync.dma_start(wxp[:], ins[f"wxp_{d}"][:])
                dt_lo = tr1.tile([R, TOK], f32, tag="dt_lo")
                bc_sb = tr1.tile([16, 2, TOK], f16, tag="bc_sb")
                for part, (m0, m1) in enumerate([(0, 48), (48, 64), (64, 80)]):
                    for ng in range(2):
                        ps = psA.tile([128, 384], f32, tag="psA")
                        for kt in range(NT):
                            nc.tensor.matmul(
                                ps[:m1 - m0, :],
                                wxp[:, kt, m0:m1],
                                xc[:, kt, ng * 384:(ng + 1) * 384],
                                start=(kt == 0),
                                stop=(kt == NT - 1),
                            )
                        if part == 0:
                            nc.scalar.copy(dt_lo[:, ng * 384:(ng + 1) * 384], ps[:R, :])
                        else:
                            nc.scalar.copy(
                                bc_sb[:, part - 1, ng * 384:(ng + 1) * 384],
                                ps[:16, :],
                            )

                # stage B/C to DRAM per half (contiguous (n,b,t))
                bstage = dramp.tile([NH, HV], f16, tag="bstage")
                cstage = dramp.tile([NH, HV], f16, tag="cstage")
                for part, stg in ((0, bstage), (1, cstage)):
                    for h in range(NH):
                        nc.sync.dma_start(
                            stg[h, :].rearrange("(n a) -> n a", n=16),
                            bc_sb[:, part, h * HL:(h + 1) * HL],
                        )

                dbg(f"dtlo_{d}", dt_lo[:])
                dbg(f"bc_{d}", bc_sb[:])
                wdt = wpool.tile([R, DI], f32, tag="wdt")
                nc.sync.dma_start(wdt[:], ins[f"wdt_{d}"][:])

                ygated = persist.tile([128, NT, TOK], f16, tag="ygated")

                # ---------------- phase 3: half-outer scan ------------------
                for h in range(NH):
                    brep = repsp.tile([128, HV], f16, tag="brep")
                    sa = bstage[h, :]
                    nc.sync.dma_start(
                        brep[:],
                        AP(tensor=sa.tensor, offset=sa.offset, ap=[[0, 128], [1, HV]]),
                    )
                    crep = repsp.tile([128, HV], f16, tag="crep")
                    sc = cstage[h, :]
                    nc.sync.dma_start(
                        crep[:],
                        AP(tensor=sc.tensor, offset=sc.offset, ap=[[0, 128], [1, HV]]),
                    )
                    crep_v = crep[:]

                    for mt in range(NT):
                        # dt = softplus(wdt.T @ dt_lo + dt_b), this half only
                        dt_t = tr2.tile([128, BH, L], f32, tag="fbl32")
                        ps = psA.tile([128, 384], f32, tag="psA")
                        nc.tensor.matmul(
                            ps[:], wdt[:, mt * 128:(mt + 1) * 128],
                            dt_lo[:, h * HL:(h + 1) * HL],
                            start=True, stop=True,
                        )
                        dt_f = dt_t[:].rearrange("p b l -> p (b l)")
                        nc.scalar.activation(
                            out=dt_f, in_=ps[:], func=AF.Exp,
                            bias=aux[d][:, mt, 5:6],
                        )
                        nc.scalar.activation(
                            out=dt_f, in_=dt_f, func=AF.Ln, bias=1.0,
                        )
                        xc_h = xc[:, mt, h * HL:(h + 1) * HL].rearrange(
                            "p (b l) -> p b l", l=L)
                        if mt == 0:
                            dbg(f"dt_{d}_h{h}", dt_t[:])
                        wt_t = tr2.tile([128, BH, L], f16, tag="wt_t")
                        nc.vector.tensor_tensor(
                            out=wt_t[:], in0=dt_t[:], in1=xc_h, op=OP.mult,
                        )
                        nc.vector.memset(dt_t[:, :, 0:1], 1e30)

                        powt = scanp.tile([128, NS, BH, L], f16, tag="powt")
                        for n in range(NS):
                            nc.scalar.activation(
                                out=powt[:, n, :, :], in_=dt_t[:],
                                func=AF.Exp, scale=-(float(n + 1)),
                            )
                        if mt == 0:
                            dbg(f"pow_{d}_h{h}", powt[:])
                        wbt = scanp.tile([128, NS, BH, L], f16, tag="scr12")
                        wt_v = wt_t[:]
                        wt_bc = AP(
                            tensor=wt_v.tensor, offset=wt_v.offset,
                            ap=[wt_v.ap[0], [0, NS], wt_v.ap[1], wt_v.ap[2]],
                        )
                        nc.vector.tensor_tensor(
                            out=wbt[:], in0=wt_bc,
                            in1=brep[:].rearrange("p (n b l) -> p n b l", n=NS, l=L),
                            op=OP.mult,
                        )
                        if mt == 0:
                            dbg(f"wb_{d}_h{h}", wbt[:])
                        nc.vector.tensor_tensor_scan(
                            out=powt[:].rearrange("p n b l -> p (n b l)"),
                            data0=powt[:].rearrange("p n b l -> p (n b l)"),
                            data1=wbt[:].rearrange("p n b l -> p (n b l)"),
                            initial=0.0,
                            op0=OP.mult, op1=OP.add,
                        )

                        if mt == 0:
                            dbg(f"h_{d}_h{h}", powt[:])
                        yfin = tr2.tile([128, BH, L], f32, tag="yfin")
                        yf_f = yfin[:].rearrange("p b l -> p (b l)")
                        hap = powt[:]
                        for q in range(NQ):
                            off = q * QBT
                            h_btn = AP(
                                tensor=hap.tensor, offset=hap.offset + off,
                                ap=[hap.ap[0], [1, QBT], [HL, NS]],
                            )
                            c_btn = AP(
                                tensor=crep_v.tensor, offset=crep_v.offset + off,
                                ap=[crep_v.ap[0], [1, QBT], [HL, NS]],
                            )
                            ycum = scanp.tile([128, QBT, NS], f32, tag="scr12")
                            nc.vector._custom_dve(
                                MULCUMSUM, out=ycum[:], in0=h_btn, in1=c_btn,
                            )
                            ycf = ycum[:].rearrange("p a n -> p (a n)")
                            nc.vector.tensor_tensor(
                                out=AP(tensor=yf_f.tensor,
                                       offset=yf_f.offset + off + 1,
                                       ap=[yf_f.ap[0], [1, QBT - 1]]),
                                in0=AP(tensor=ycf.tensor,
                                       offset=ycf.offset + 2 * NS - 1,
                                       ap=[ycf.ap[0], [NS, QBT - 1]]),
                                in1=AP(tensor=ycf.tensor,
                                       offset=ycf.offset + NS - 1,
                                       ap=[ycf.ap[0], [NS, QBT - 1]]),
                                op=OP.subtract,
                            )
                            nc.vector.tensor_copy(
                                AP(tensor=yf_f.tensor, offset=yf_f.offset + off,
                                   ap=[yf_f.ap[0], [1, 1]]),
                                AP(tensor=ycf.tensor, offset=ycf.offset + NS - 1,
                                   ap=[ycf.ap[0], [1, 1]]),
                            )

                        # skip term + write token-ordered into ygated slot
                        og = ygated[:, mt, h * HL:(h + 1) * HL].rearrange(
                            "p (b l) -> p b l", l=L)
                        if not fwd:
                            og = AP(tensor=og.tensor, offset=og.offset + 5,
                                    ap=[og.ap[0], og.ap[1], [-1, L]])
                        nc.vector.scalar_tensor_tensor(
                            out=og, in0=xc_h, scalar=aux[d][:, mt, 6:7],
                            in1=yfin[:], op0=OP.mult, op1=OP.add,
                        )

                # ---------------- phase 3b: z-silu gate --------------------
                for mt in range(NT):
                    wz_t = wstream.tile([128, KT, 128], f16, tag="wz_t")
                    nc.sync.dma_start(wz_t[:], ins[f"wz_{d}"][mt, :, :, :])
                    for ng in range(2):
                        ps2 = psA.tile([128, 384], f32, tag="psA")
                        for kt in range(KT):
                            nc.tensor.matmul(
                                ps2[:], wz_t[:, kt, :],
                                xT[:, kt, ng * 384:(ng + 1) * 384],
                                start=(kt == 0), stop=(kt == KT - 1),
                            )
                        sg_t = tr2.tile([128, 384], f16, tag="sg_t")
                        nc.scalar.activation(out=sg_t[:], in_=ps2[:], func=AF.Sigmoid)
                        sz_t = tr2.tile([128, 384], f16, tag="sz_t")
                        nc.vector.scalar_tensor_tensor(
                            out=sz_t[:], in0=ps2[:], scalar=1.0, in1=sg_t[:],
                            op0=OP.mult, op1=OP.mult,
                        )
                        o = ygated[:, mt, ng * 384:(ng + 1) * 384]
                        nc.vector.tensor_tensor(
                            out=o, in0=o, in1=sz_t[:], op=OP.mult,
                        )

                dbg(f"ygated_{d}", ygated[:])
                # ---------------- phase 4: out_proj ------------------------
                for ng in range(2):
                    for mg in range(2):
                        pso = [psO.tile([128, 384], f32, tag=f"psO{m}", name=f"psO{m}")
                               for m in range(3)]
                        for kt in range(NT):
                            wo_t = wstream.tile([128, 3, 128], f16, tag="wo_t")
                            nc.sync.dma_start(
                                wo_t[:],
                                dram_ap(
                                    ins[f"wout_{d}"],
                                    kt * 128 * D + mg * 384,
                                    [[D, 128], [128, 3], [1, 128]],
                                ),
                            )
                            for m in range(3):
                                nc.tensor.matmul(
                                    pso[m][:], wo_t[:, m, :],
                                    ygated[:, kt, ng * 384:(ng + 1) * 384],
                                    start=(kt == 0), stop=(kt == NT - 1),
                                )
                        for m in range(3):
                            mt_e = mg * 3 + m
                            o = yout[:, mt_e, ng * 384:(ng + 1) * 384]
                            if dir_i == 0:
                                nc.scalar.copy(o, pso[m][:])
                            else:
                                nc.vector.tensor_tensor(
                                    out=o, in0=o, in1=pso[m][:], op=OP.add
                                )

            dbg("yout", yout[:])
            # ---------------- phase 5: residual + LN -> out --------------
            for tt in range(ET):
                xtok = tr1.tile([128, D], f32, tag="xtok")
                nc.sync.dma_start(xtok[:], x_d[tt * 128:(tt + 1) * 128, :])
                r_t = tr1.tile([128, D], f32, tag="r_t")
                for ec in range(KT):
                    psh = psT.tile([128, 128], f16, tag="pst")
                    nc.tensor.transpose(
                        psh[:], yout[:, ec, tt * 128:(tt + 1) * 128], identh[:]
                    )
                    nc.vector.tensor_tensor(
                        out=r_t[:, ec * 128:(ec + 1) * 128],
                        in0=psh[:], in1=xtok[:, ec * 128:(ec + 1) * 128],
                        op=OP.add,
                    )
                stats = tr1.tile([128, 3, nc.vector.BN_STATS_DIM], f32, tag="stats")
                for sub in range(3):
                    nc.vector.bn_stats(
                        out=stats[:, sub, :], in_=r_t[:, sub * 256:(sub + 1) * 256]
                    )
                mv = tr1.tile([128, nc.vector.BN_AGGR_DIM], f32, tag="mv")
                nc.vector.bn_aggr(out=mv[:], in_=stats[:])
                rstd = tr1.tile([128, 1], f32, tag="rstd")
                nc.scalar.activation(
                    out=rstd[:], in_=mv[:, 1:2], func=AF.Sqrt, bias=eps_t[:],
                )
                nc.vector.reciprocal(out=rstd[:], in_=rstd[:])
                nc.vector.tensor_scalar(
                    out=r_t[:], in0=r_t[:], scalar1=mv[:, 0:1], scalar2=rstd[:],
                    op0=OP.subtract, op1=OP.mult,
                )
                nc.vector.tensor_tensor(out=r_t[:], in0=r_t[:], in1=g_rep[:], op=OP.mult)
                nc.vector.tensor_tensor(out=r_t[:], in0=r_t[:], in1=b_rep[:], op=OP.add)
                nc.sync.dma_start(out_d[tt * 128:(tt + 1) * 128, :], r_t[:])

    nc.compile()
    return nc


def _prep_inputs(inputs):
    f16 = np.float16
    shared = {}
    for d in ("f", "b"):
        in_proj = np.asarray(inputs[f"{d}_in"], np.float32)      # [3072, 768]
        shared[f"win_{d}"] = np.ascontiguousarray(in_proj[:DI].T).astype(f16)
        wz_T = in_proj[DI:].T                                    # [768, 1536]
        shared[f"wz_{d}"] = np.ascontiguousarray(
            wz_T.reshape(KT, 128, NT, 128).transpose(2, 1, 0, 3)
        ).astype(f16)
        xp_T = np.asarray(inputs[f"{d}_xp"], np.float32).T       # [1536, 80]
        shared[f"wxp_{d}"] = np.ascontiguousarray(
            xp_T.reshape(NT, 128, 80).transpose(1, 0, 2)
        ).astype(f16)
        shared[f"wdt_{d}"] = np.ascontiguousarray(
            np.asarray(inputs[f"{d}_dtw"], np.float32).T
        ).astype(np.float32)                                     # [48, 1536]
        shared[f"wout_{d}"] = np.ascontiguousarray(
            np.asarray(inputs[f"{d}_out"], np.float32).T
        ).astype(f16)                                            # [1536, 768]
        aux = np.zeros((DI, 8), np.float32)
        aux[:, 0:4] = np.asarray(inputs[f"{d}_cw"], np.float32).T
        aux[:, 4] = np.asarray(inputs[f"{d}_cb"], np.float32)
        aux[:, 5] = np.asarray(inputs[f"{d}_dtb"], np.float32)
        aux[:, 6] = np.asarray(inputs[f"{d}_D"], np.float32)
        shared[f"aux_{d}"] = aux
    shared["ln_g"] = np.ascontiguousarray(np.asarray(inputs["ln_g"], np.float32))
    shared["ln_b"] = np.ascontiguousarray(np.asarray(inputs["ln_b"], np.float32))
    return shared


def kernel(**inputs):
    from concourse import bass_utils

    if "nc" not in _CACHE:
        _CACHE["nc"] = _build_module()
    nc = _CACHE["nc"]

    shared = _prep_inputs(inputs)
    x = np.asarray(inputs["x"], np.float32)
    n_cores = 8
    bs = x.shape[0] // n_cores

    in_maps = []
    for c in range(n_cores):
        m = dict(shared)
        m["x"] = np.ascontiguousarray(
            x[c * bs:(c + 1) * bs].reshape(TOK, D)
        ).astype(np.float32)
        in_maps.append(m)

    res = bass_utils.run_bass_kernel_spmd(nc, in_maps, core_ids=list(range(n_cores)))
    out = np.concatenate(
        [r["out"].reshape(bs, L, D) for r in res.results], axis=0
    )
    return out.astype(np.float32)



# revision 7
# speedup vs baseline: 1.2264x; 1.2264x over previous
"""Bidirectional Mamba (PartContextMamba) Trainium2 Bass kernel, v2.

Sharding: pure data parallelism over batch (1024 -> 8 cores x 128 batch).
Token order on all free axes is (l, b) -- l OUTER, b inner (host reshapes
x to l-major). This makes every scan-phase access contiguous:

  xT [768d, (l b)] f16 (PE transpose of the x shard)
  per direction (fwd, bwd):
    xi = W_in_xi @ xT (PE), conv via shifted-slab STTs (DVE), silu (ACT)
    x_dbl = W_xp @ xc -> dt_lo[48] f32, B[16], C[16] f16
    B/C staged to DRAM as (half, t, n, b) then partition-broadcast to all
    128 partitions (brep/crep).
    per mt (12 d-tiles of 128):
      dt = softplus(W_dt @ dt_lo + dt_b)  (ACT Exp+Ln, one table)
      wt = dt*xc (DVE)
      per b-half h (64):
        pow[n,t,b] = exp(-(n+1)dt): 8 ACT exps + 1 DVE doubling TT
        wbh[t,n,b] = wt x brep (DVE TT, becomes h in place)
        recurrence h[t] = pow[t]*h[t-1] + wbh[t]: 10 unrolled TTs
        (in-place over wbh; bwd runs the slab loop in reverse)
        hc = h*crep -> pow buffer; log-tree reduce over n (GpSimd TTs,
        ping-pong between wbh/pow buffers) -> yfin f32
        y = yfin + D*xc (STT) -> ygated slot
    z-gate: ygated *= silu(W_z @ xT) (PE + ACT Silu + DVE TT)
    yout += W_out @ ygated (PE, PSUM k-accum)
  out = LayerNorm(x + yout^T) (PE transpose, ACT Rsqrt, token-major)
"""

import numpy as np

_CACHE: dict = {}

B = 128          # batch per core
L = 6
D = 768
DI = 1536
NT = 12          # d-tiles
NS = 16          # ssm states
R = 48           # dt rank
TOK = B * L
ET = 6           # token-tiles (now l-slabs)
KT = 6           # k-tiles of D
NH = 2           # b-halves
BH = B // NH     # 64
HV = NS * BH * L  # 6144 elems per half of brep/crep
SL = NS * BH     # 1024, one t-slab in (t,n,b)


def _build_module(debug=False):
    import concourse.bass as bass
    import concourse.bacc as bacc
    import concourse.mybir as mybir
    import concourse.tile as tile
    from concourse.masks import make_identity

    f32 = mybir.dt.float32
    f16 = mybir.dt.float16
    AP = bass.AP
    AF = mybir.ActivationFunctionType
    OP = mybir.AluOpType

    nc = bacc.Bacc("TRN2", target_bir_lowering=False)

    x_d = nc.dram_tensor("x", [TOK, D], f32, kind="ExternalInput")
    ins = {}
    for d in ("f", "b"):
        ins[f"win_{d}"] = nc.dram_tensor(f"win_{d}", [D, DI], f16, kind="ExternalInput")
        ins[f"wz_{d}"] = nc.dram_tensor(f"wz_{d}", [NT, 128, KT, 128], f16, kind="ExternalInput")
        ins[f"wxp_{d}"] = nc.dram_tensor(f"wxp_{d}", [128, NT, 80], f16, kind="ExternalInput")
        ins[f"wdt_{d}"] = nc.dram_tensor(f"wdt_{d}", [R, DI], f16, kind="ExternalInput")
        ins[f"wout_{d}"] = nc.dram_tensor(f"wout_{d}", [DI, D], f16, kind="ExternalInput")
        ins[f"aux_{d}"] = nc.dram_tensor(f"aux_{d}", [DI, 8], f32, kind="ExternalInput")
    lng_d = nc.dram_tensor("ln_g", [D], f32, kind="ExternalInput")
    lnb_d = nc.dram_tensor("ln_b", [D], f32, kind="ExternalInput")
    out_d = nc.dram_tensor("out", [TOK, D], f32, kind="ExternalOutput")

    def dram_ap(t, offset, ap):
        return AP(tensor=t, offset=offset, ap=ap)

    def dbg(name, ap):
        if not debug:
            return
        p = ap.partition_size()
        counts = [c for _, c in ap.ap[1:]]
        t = nc.dram_tensor(f"dbg_{name}", [p] + counts, ap.dtype,
                           kind="ExternalOutput")
        nc.sync.dma_start(t[:], ap)

    with tile.TileContext(nc) as tc:
        with (
            tc.tile_pool(name="consts", bufs=1) as consts,
            tc.tile_pool(name="persist", bufs=1) as persist,
            tc.tile_pool(name="wpool", bufs=1) as wpool,
            tc.tile_pool(name="wstream", bufs=2) as wstream,
            tc.tile_pool(name="tr2", bufs=2) as tr2,
            tc.tile_pool(name="tr1", bufs=1) as tr1,
            tc.tile_pool(name="scanp", bufs=2) as scanp,
            tc.tile_pool(name="reps", bufs=1) as repsp,
            tc.tile_pool(name="dram", bufs=1, space="DRAM") as dramp,
            tc.tile_pool(name="psA", bufs=2, space="PSUM") as psA,
            tc.tile_pool(name="psT", bufs=1, space="PSUM") as psT,
            tc.tile_pool(name="psO", bufs=1, space="PSUM") as psO,
        ):
            # ---------------- constants ----------------
            ident = consts.tile([128, 128], f32)
            make_identity(nc, ident)
            identh = consts.tile([128, 128], f16)
            nc.vector.tensor_copy(identh[:], ident[:])
            g_rep = consts.tile([128, D], f32)
            nc.sync.dma_start(g_rep[:], dram_ap(lng_d, 0, [[0, 128], [1, D]]))
            b_rep = consts.tile([128, D], f32)
            nc.sync.dma_start(b_rep[:], dram_ap(lnb_d, 0, [[0, 128], [1, D]]))
            eps_t = consts.tile([128, 1], f32)
            nc.vector.memset(eps_t[:], 1e-5)
            aux = {}
            for d in ("f", "b"):
                aux[d] = consts.tile([128, NT, 8], f32, tag=f"aux_{d}", name=f"aux_{d}")
                nc.sync.dma_start(
                    aux[d][:],
                    dram_ap(ins[f"aux_{d}"], 0, [[8, 128], [8 * 128, NT], [1, 8]]),
                )

            # ---------------- xT (fp16) via PE transpose ----------------
            # x_d rows are tokens in (l, b) order (host reshaped l-major),
            # so chunk tt == l-slab tt.
            xT = persist.tile([128, KT, TOK], f16, tag="xT")
            for tt in range(ET):
                xtok = tr1.tile([128, D], f32, tag="xtok")
                nc.sync.dma_start(xtok[:], x_d[tt * 128:(tt + 1) * 128, :])
                for ec in range(KT):
                    pst = psT.tile([128, 128], f32, tag="pst")
                    nc.tensor.transpose(pst[:], xtok[:, ec * 128:(ec + 1) * 128], ident[:])
                    nc.scalar.copy(xT[:, ec, tt * 128:(tt + 1) * 128], pst[:])

            dbg("xT", xT[:])
            yout = persist.tile([128, ET, TOK], f16, tag="yout")
            xc = persist.tile([128, NT, TOK], f16, tag="xc")

            for dir_i, d in enumerate(("f", "b")):
                fwd = d == "f"

                # ---------------- phase 1: in_proj + conv -> xc ------------
                for mt in range(NT):
                    win_t = wstream.tile([128, KT, 128], f16, tag="win_t")
                    for kt in range(KT):
                        nc.sync.dma_start(
                            win_t[:, kt, :],
                            dram_ap(ins[f"win_{d}"],
                                    kt * 128 * DI + mt * 128,
                                    [[DI, 128], [1, 128]]),
                        )
                    ps = psA.tile([128, 2, 512], f32, tag="psA")
                    for kt in range(KT):
                        for ng in range(2):
                            nc.tensor.matmul(
                                ps[:, ng, 0:384],
                                win_t[:, kt, :],
                                xT[:, kt, ng * 384:(ng + 1) * 384],
                                start=(kt == 0),
                                stop=(kt == KT - 1),
                            )
                    xi_t = tr2.tile([128, TOK], f16, tag="scr16")
                    for ng in range(2):
                        nc.scalar.copy(xi_t[:, ng * 384:(ng + 1) * 384],
                                       ps[:, ng, 0:384])

                    # conv: (l,b) layout, shifted-slab accumulation
                    acc = tr2.tile([128, TOK], f32, tag="scr32")
                    cw = [aux[d][:, mt, k:k + 1] for k in range(4)]
                    if fwd:
                        # xc[l] = sum_k w[k] * xi[l+k-3]
                        nc.vector.tensor_scalar(
                            out=acc[:], in0=xi_t[:], scalar1=cw[3], scalar2=None,
                            op0=OP.mult,
                        )
                        for k in range(3):
                            off = (3 - k) * 128
                            nc.vector.scalar_tensor_tensor(
                                out=acc[:, off:TOK], in0=xi_t[:, 0:TOK - off],
                                scalar=cw[k],
                                in1=acc[:, off:TOK], op0=OP.mult, op1=OP.add,
                            )
                    else:
                        # xc[l] = sum_j w[3-j] * xi[l+j]
                        nc.vector.tensor_scalar(
                            out=acc[:], in0=xi_t[:], scalar1=cw[3], scalar2=None,
                            op0=OP.mult,
                        )
                        for j in range(1, 4):
                            off = j * 128
                            nc.vector.scalar_tensor_tensor(
                                out=acc[:, 0:TOK - off], in0=xi_t[:, off:TOK],
                                scalar=cw[3 - j],
                                in1=acc[:, 0:TOK - off], op0=OP.mult, op1=OP.add,
                            )
                    nc.scalar.activation(
                        out=xc[:, mt, :], in_=acc[:], func=AF.Silu,
                        bias=aux[d][:, mt, 4:5],
                    )

                dbg(f"xc_{d}", xc[:])
                # ---------------- phase 2: x_proj -> dt_lo, B, C -----------
                wxp = wpool.tile([128, NT, 80], f16, tag="wxp")
                nc.sync.dma_start(wxp[:], ins[f"wxp_{d}"][:])
                dt_lo = tr1.tile([R, TOK], f16, tag="dt_lo")
                bc_sb = tr1.tile([16, 2, TOK], f16, tag="bc_sb")
                for part, (m0, m1) in enumerate([(0, 48), (48, 64), (64, 80)]):
                    psx = psA.tile([128, 2, 512], f32, tag="psA")
                    for kt in range(NT):
                        for ng in range(2):
                            nc.tensor.matmul(
                                psx[:m1 - m0, ng, 0:384],
                                wxp[:, kt, m0:m1],
                                xc[:, kt, ng * 384:(ng + 1) * 384],
                                start=(kt == 0),
                                stop=(kt == NT - 1),
                            )
                    for ng in range(2):
                        if part == 0:
                            nc.scalar.copy(dt_lo[:, ng * 384:(ng + 1) * 384],
                                           psx[:R, ng, 0:384])
                        else:
                            nc.scalar.copy(
                                bc_sb[:, part - 1, ng * 384:(ng + 1) * 384],
                                psx[:16, ng, 0:384],
                            )

                # stage B/C to DRAM as (half, t, n, b) then broadcast-read
                bstage = dramp.tile([NH, L, NS, BH], f16, tag="bstage")
                cstage = dramp.tile([NH, L, NS, BH], f16, tag="cstage")
                for part, stg in ((0, bstage), (1, cstage)):
                    for h in range(NH):
                        for t in range(L):
                            nc.sync.dma_start(
                                stg[h, t, :, :],
                                bc_sb[:, part, t * 128 + h * BH:
                                      t * 128 + h * BH + BH],
                            )
                brep = repsp.tile([128, NH, L, NS, BH], f16, tag="brep")
                sa = bstage[:, :, :, :]
                nc.sync.dma_start(
                    brep[:],
                    AP(tensor=sa.tensor, offset=sa.offset,
                       ap=[[0, 128], [1, NH * HV]]),
                )
                crep = repsp.tile([128, NH, L, NS, BH], f16, tag="crep")
                sc = cstage[:, :, :, :]
                nc.sync.dma_start(
                    crep[:],
                    AP(tensor=sc.tensor, offset=sc.offset,
                       ap=[[0, 128], [1, NH * HV]]),
                )

                dbg(f"dtlo_{d}", dt_lo[:])
                dbg(f"bc_{d}", bc_sb[:])
                wdt = wpool.tile([R, DI], f16, tag="wdt")
                nc.sync.dma_start(wdt[:], ins[f"wdt_{d}"][:])

                ygated = persist.tile([128, NT, TOK], f16, tag="ygated")

                # ---------------- phase 3: scan ------------------
                for mt in range(NT):
                    # dt = softplus(wdt.T @ dt_lo + dt_b), all tokens
                    psd = psA.tile([128, 2, 512], f32, tag="psA")
                    for ng in range(2):
                        nc.tensor.matmul(
                            psd[:, ng, 0:384], wdt[:, mt * 128:(mt + 1) * 128],
                            dt_lo[:, ng * 384:(ng + 1) * 384],
                            start=True, stop=True,
                        )
                    dte = tr2.tile([128, TOK], f32, tag="scr32")
                    for ng in range(2):
                        nc.scalar.activation(
                            out=dte[:, ng * 384:(ng + 1) * 384],
                            in_=psd[:, ng, 0:384], func=AF.Exp,
                            bias=aux[d][:, mt, 5:6],
                        )
                    dt16 = tr2.tile([128, TOK], f16, tag="dt16")
                    nc.scalar.activation(
                        out=dt16[:], in_=dte[:], func=AF.Ln, bias=1.0,
                    )
                    if mt == 0:
                        dbg(f"dt_{d}", dt16[:])
                    # wt = dt * xc, (l,b) f16
                    wt_t = tr2.tile([128, TOK], f16, tag="wt_t")
                    nc.vector.tensor_tensor(
                        out=wt_t[:], in0=dt16[:], in1=xc[:, mt, :], op=OP.mult,
                    )

                    for h in range(NH):
                        # pow[n, (t b)] = exp(-(n+1)*dt) for this b-half
                        powt = scanp.tile([128, NS, L, BH], f16, tag="powt")
                        dt_h = AP(tensor=dt16[:].tensor,
                                  offset=dt16[:].offset + h * BH,
                                  ap=[dt16[:].ap[0], [128, L], [1, BH]])
                        for n in range(8):
                            nc.scalar.activation(
                                out=powt[:, n, :, :], in_=dt_h,
                                func=AF.Exp, scale=-(float(n + 1)),
                            )
                        p8 = powt[:, 7, :, :]
                        nc.vector.tensor_tensor(
                            out=powt[:, 8:16, :, :],
                            in0=powt[:, 0:8, :, :],
                            in1=AP(tensor=p8.tensor, offset=p8.offset,
                                   ap=[p8.ap[0], [0, 8], [1, L * BH]]),
                            op=OP.mult,
                        )
                        if mt == 0:
                            dbg(f"pow_{d}_h{h}", powt[:])

                        # wbh[t, n, b] = wt x brep; becomes h in place
                        wbh = scanp.tile([128, L, NS, BH], f16, tag="wbh")
                        wt_v = wt_t[:]
                        nc.vector.tensor_tensor(
                            out=wbh[:],
                            in0=AP(tensor=wt_v.tensor,
                                   offset=wt_v.offset + h * BH,
                                   ap=[wt_v.ap[0], [128, L], [0, NS], [1, BH]]),
                            in1=brep[:, h, :, :, :],
                            op=OP.mult,
                        )
                        if mt == 0:
                            dbg(f"wb_{d}_h{h}", wbh[:])

                        # recurrence, in place: h[t] = pow[t]*h[prev] + wbh[t]
                        steps = range(1, L) if fwd else range(L - 2, -1, -1)
                        for t in steps:
                            tprev = t - 1 if fwd else t + 1
                            tmp = scanp.tile([128, SL], f16, tag="tmp")
                            pv = powt[:]
                            pow_t = AP(
                                tensor=pv.tensor, offset=pv.offset + t * BH,
                                ap=[pv.ap[0], [L * BH, NS], [1, BH]])
                            nc.vector.tensor_tensor(
                                out=tmp[:], in0=pow_t,
                                in1=wbh[:, tprev, :, :].rearrange(
                                    "p n b -> p (n b)"),
                                op=OP.mult,
                            )
                            nc.vector.tensor_tensor(
                                out=wbh[:, t, :, :].rearrange("p n b -> p (n b)"),
                                in0=tmp[:],
                                in1=wbh[:, t, :, :].rearrange("p n b -> p (n b)"),
                                op=OP.add,
                            )
                        if mt == 0:
                            dbg(f"h_{d}_h{h}", wbh[:])

                        # hc = h * crep -> pow buffer (as (t,n,b) scratch)
                        hcb = powt[:].rearrange("p n t b -> p (n t b)")
                        nc.vector.tensor_tensor(
                            out=hcb,
                            in0=wbh[:].rearrange("p t n b -> p (t n b)"),
                            in1=crep[:, h, :, :, :].rearrange(
                                "p t n b -> p (t n b)"),
                            op=OP.mult,
                        )
                        # log-tree reduce over n (gpsimd), ping-pong buffers
                        def lvl(src, dst, n_out):
                            w = n_out * BH
                            i0 = AP(tensor=src.tensor, offset=src.offset,
                                    ap=[src.ap[0], [SL, L], [1, w]])
                            i1 = AP(tensor=src.tensor, offset=src.offset + w,
                                    ap=[src.ap[0], [SL, L], [1, w]])
                            o = AP(tensor=dst.tensor, offset=dst.offset,
                                   ap=[dst.ap[0], [SL, L], [1, w]])
                            nc.gpsimd.tensor_tensor(out=o, in0=i0, in1=i1,
                                                    op=OP.add)

                        lvl(hcb, wbh[:].rearrange("p t n b -> p (t n b)"), 8)
                        lvl(wbh[:].rearrange("p t n b -> p (t n b)"), hcb, 4)
                        lvl(hcb, wbh[:].rearrange("p t n b -> p (t n b)"), 2)
                        # final level -> f32 yfin [L, BH]
                        yfin = scanp.tile([128, L, BH], f32, tag="yfin")
                        wbf = wbh[:].rearrange("p t n b -> p (t n b)")
                        nc.gpsimd.tensor_tensor(
                            out=yfin[:].rearrange("p t b -> p (t b)"),
                            in0=AP(tensor=wbf.tensor, offset=wbf.offset,
                                   ap=[wbf.ap[0], [SL, L], [1, BH]]),
                            in1=AP(tensor=wbf.tensor, offset=wbf.offset + BH,
                                   ap=[wbf.ap[0], [SL, L], [1, BH]]),
                            op=OP.add,
                        )
                        if mt == 0:
                            dbg(f"yfin_{d}_h{h}", yfin[:])

                        # skip term: ygated slot = D*xc + yfin, (l, b-half)
                        og = ygated[:, mt, :]
                        nc.vector.scalar_tensor_tensor(
                            out=AP(tensor=og.tensor, offset=og.offset + h * BH,
                                   ap=[og.ap[0], [128, L], [1, BH]]),
                            in0=AP(tensor=xc[:, mt, :].tensor,
                                   offset=xc[:, mt, :].offset + h * BH,
                                   ap=[og.ap[0], [128, L], [1, BH]]),
                            scalar=aux[d][:, mt, 6:7],
                            in1=yfin[:].rearrange("p t b -> p (t b)"),
                            op0=OP.mult, op1=OP.add,
                        )

                # ---------------- phase 3b: z-silu gate --------------------
                for mt in range(NT):
                    wz_t = wstream.tile([128, KT, 128], f16, tag="wz_t")
                    nc.sync.dma_start(wz_t[:], ins[f"wz_{d}"][mt, :, :, :])
                    psz = psA.tile([128, 2, 512], f32, tag="psA")
                    for kt in range(KT):
                        for ng in range(2):
                            nc.tensor.matmul(
                                psz[:, ng, 0:384], wz_t[:, kt, :],
                                xT[:, kt, ng * 384:(ng + 1) * 384],
                                start=(kt == 0), stop=(kt == KT - 1),
                            )
                    sz_t = tr2.tile([128, TOK], f16, tag="scr16")
                    for ng in range(2):
                        nc.scalar.activation(
                            out=sz_t[:, ng * 384:(ng + 1) * 384],
                            in_=psz[:, ng, 0:384], func=AF.Silu)
                    o = ygated[:, mt, :]
                    nc.vector.tensor_tensor(out=o, in0=o, in1=sz_t[:], op=OP.mult)

                dbg(f"ygated_{d}", ygated[:])
                # ---------------- phase 4: out_proj ------------------------
                for ng in range(2):
                    for mg in range(2):
                        pso = psO.tile([128, 3, 512], f32, tag="psO")
                        for kt in range(NT):
                            wo_t = wstream.tile([128, 3, 128], f16, tag="wo_t")
                            nc.sync.dma_start(
                                wo_t[:],
                                dram_ap(
                                    ins[f"wout_{d}"],
                                    kt * 128 * D + mg * 384,
                                    [[D, 128], [128, 3], [1, 128]],
                                ),
                            )
                            for m in range(3):
                                nc.tensor.matmul(
                                    pso[:, m, 0:384], wo_t[:, m, :],
                                    ygated[:, kt, ng * 384:(ng + 1) * 384],
                                    start=(kt == 0), stop=(kt == NT - 1),
                                )
                        for m in range(3):
                            mt_e = mg * 3 + m
                            o = yout[:, mt_e, ng * 384:(ng + 1) * 384]
                            if dir_i == 0:
                                nc.scalar.copy(o, pso[:, m, 0:384])
                            else:
                                nc.vector.tensor_tensor(
                                    out=o, in0=o, in1=pso[:, m, 0:384], op=OP.add)

            dbg("yout", yout[:])
            # ---------------- phase 5: residual + LN -> out --------------
            for tt in range(ET):
                xtok = tr1.tile([128, D], f32, tag="xtok")
                nc.sync.dma_start(xtok[:], x_d[tt * 128:(tt + 1) * 128, :])
                r_t = tr1.tile([128, D], f32, tag="r_t")
                for ec in range(KT):
                    psh = psT.tile([128, 128], f16, tag="pst")
                    nc.tensor.transpose(
                        psh[:], yout[:, ec, tt * 128:(tt + 1) * 128], identh[:]
                    )
                    nc.vector.tensor_tensor(
                        out=r_t[:, ec * 128:(ec + 1) * 128],
                        in0=psh[:], in1=xtok[:, ec * 128:(ec + 1) * 128],
                        op=OP.add,
                    )
                stats = tr1.tile([128, 3, nc.vector.BN_STATS_DIM], f32, tag="stats")
                for sub in range(3):
                    nc.vector.bn_stats(
                        out=stats[:, sub, :], in_=r_t[:, sub * 256:(sub + 1) * 256]
                    )
                mv = tr1.tile([128, nc.vector.BN_AGGR_DIM], f32, tag="mv")
                nc.vector.bn_aggr(out=mv[:], in_=stats[:])
                rstd = tr1.tile([128, 1], f32, tag="rstd")
                nc.scalar.activation(
                    out=rstd[:], in_=mv[:, 1:2], func=AF.Sqrt, bias=eps_t[:],
                )
                nc.vector.reciprocal(out=rstd[:], in_=rstd[:])
                nc.vector.tensor_scalar(
                    out=r_t[:], in0=r_t[:], scalar1=mv[:, 0:1], scalar2=rstd[:],
                    op0=OP.subtract, op1=OP.mult,
                )
                nc.vector.tensor_tensor(out=r_t[:], in0=r_t[:], in1=g_rep[:], op=OP.mult)
                nc.vector.tensor_tensor(out=r_t[:], in0=r_t[:], in1=b_rep[:], op=OP.add)
                nc.sync.dma_start(out_d[tt * 128:(tt + 1) * 128, :], r_t[:])

    nc.compile()
    return nc


def _prep_inputs(inputs):
    f16 = np.float16
    shared = {}
    for d in ("f", "b"):
        in_proj = np.asarray(inputs[f"{d}_in"], np.float32)      # [3072, 768]
        shared[f"win_{d}"] = np.ascontiguousarray(in_proj[:DI].T).astype(f16)
        wz_T = in_proj[DI:].T                                    # [768, 1536]
        shared[f"wz_{d}"] = np.ascontiguousarray(
            wz_T.reshape(KT, 128, NT, 128).transpose(2, 1, 0, 3)
        ).astype(f16)
        xp_T = np.asarray(inputs[f"{d}_xp"], np.float32).T       # [1536, 80]
        shared[f"wxp_{d}"] = np.ascontiguousarray(
            xp_T.reshape(NT, 128, 80).transpose(1, 0, 2)
        ).astype(f16)
        shared[f"wdt_{d}"] = np.ascontiguousarray(
            np.asarray(inputs[f"{d}_dtw"], np.float32).T
        ).astype(f16)                                            # [48, 1536]
        shared[f"wout_{d}"] = np.ascontiguousarray(
            np.asarray(inputs[f"{d}_out"], np.float32).T
        ).astype(f16)                                            # [1536, 768]
        aux = np.zeros((DI, 8), np.float32)
        aux[:, 0:4] = np.asarray(inputs[f"{d}_cw"], np.float32).T
        aux[:, 4] = np.asarray(inputs[f"{d}_cb"], np.float32)
        aux[:, 5] = np.asarray(inputs[f"{d}_dtb"], np.float32)
        aux[:, 6] = np.asarray(inputs[f"{d}_D"], np.float32)
        shared[f"aux_{d}"] = aux
    shared["ln_g"] = np.ascontiguousarray(np.asarray(inputs["ln_g"], np.float32))
    shared["ln_b"] = np.ascontiguousarray(np.asarray(inputs["ln_b"], np.float32))
    return shared


def kernel(**inputs):
    from concourse import bass_utils

    if "nc" not in _CACHE:
        _CACHE["nc"] = _build_module()
    nc = _CACHE["nc"]

    shared = _prep_inputs(inputs)
    x = np.asarray(inputs["x"], np.float32)
    n_cores = 8
    bs = x.shape[0] // n_cores

    in_maps = []
    for c in range(n_cores):
        m = dict(shared)
        # l-major token order: row t*B + b
        m["x"] = np.ascontiguousarray(
            x[c * bs:(c + 1) * bs].transpose(1, 0, 2).reshape(TOK, D)
        ).astype(np.float32)
        in_maps.append(m)

    res = bass_utils.run_bass_kernel_spmd(nc, in_maps, core_ids=list(range(n_cores)))
    out = np.concatenate(
        [r["out"].reshape(L, bs, D).transpose(1, 0, 2) for r in res.results],
        axis=0,
    )
    return out.astype(np.float32)
